# revision 17
# baseline (speedup 1.0000x reference)
"""Distributed Trainium2 kernel for nn_CEMA_34445637714419.

Math (from the reference):
    scale[d] = sum_{j,k} eta[d,j] * cos(j*omega[k]*2pi/h) * alpha[d,k] * beta[d,k]
    y[b,d]   = x[b,d] * scale[d]

The (d,) scale vector costs ~17 MFLOP — computed on host in float64.
The device kernel is the pure memory-bound part: stream x (16384,2048) f32
through SBUF, multiply by the partition-replicated scale row, stream out.
Sharding: x split along batch across 8 NeuronCores (data parallel),
scale replicated.
"""

import math

import numpy as np

try:
    import concourse.bass as bass
except ImportError:  # grading container may not have it on sys.path yet
    import sys

    sys.path.insert(0, "/opt/trn_rl_repo")
    import concourse.bass as bass

import concourse.bacc as bacc
import concourse.mybir as mybir
from concourse.bass_utils import run_bass_kernel_spmd
from concourse.tile import TileContext
from concourse.tile_rust import add_dep_helper

BATCH = 16384
D = 2048
H = 64
N_CORES = 8
SHARD = BATCH // N_CORES  # 2048 rows per core
P = 128  # SBUF partitions
N_TILES = SHARD // P  # 16 tiles of (128, 2048) = 1 MiB each


def build_nc() -> bacc.Bacc:
    nc = bacc.Bacc(
        "TRN2", target_bir_lowering=False, debug=False, num_devices=N_CORES
    )
    f32 = mybir.dt.float32
    x_ext = nc.declare_dram_parameter("x", [SHARD, D], f32, isOutput=False)
    s_ext = nc.declare_dram_parameter("scale", [P, D], f32, isOutput=False)
    out_ext = nc.declare_dram_parameter("out", [SHARD, D], f32, isOutput=True)

    with TileContext(nc) as tc:
        with (
            tc.tile_pool(name="const", bufs=1) as cpool,
            # bufs == N_TILES: every x tile gets its own SBUF slot (16 MiB
            # total), so there is no slot reuse and no WAR/WAW waits — the
            # TensorTensor ISA slot only fits one semaphore wait.
            tc.tile_pool(name="io", bufs=N_TILES) as pool,
        ):
            s_tile = cpool.tile([P, D], f32)
            scratch = cpool.tile([P, 1], f32)
            # Scale load on the ACT HWDGE ring (nc.scalar) so the SP ring
            # can start streaming x immediately.
            nc.scalar.dma_start(s_tile[:], s_ext[:])
            # Tiny DVE-side read of s_tile: absorbs the scale-DMA
            # semaphore wait into the DVE queue, so each tensor_mul below
            # needs only its own x-DMA wait (the TT ISA slot fits one wait).
            nc.vector.tensor_copy(out=scratch[:], in_=s_tile[:, 0:1])
            # Reads and writes interleave on BOTH HWDGE rings (SP + ACT),
            # with writes lagging WRITE_LAG tiles so each write's mul is
            # long done when it reaches the ring head (no sequencer stall).
            # Both rings then stream mixed traffic continuously (~425 GB/s
            # combined, the fabric cap) and finish together — instead of
            # the write stream trailing the reads by ~14 us.
            WRITE_LAG = 9
            tiles = []
            ring_order = {id(nc.sync): [], id(nc.scalar): []}

            def ring_dma(eng, out, in_):
                bi = eng.dma_start(out, in_)
                chain = ring_order[id(eng)]
                if chain:
                    # nosync edge pins the sequencer/ring order so the
                    # scheduler can't hoist a write ahead of later reads
                    # (its unsatisfied wait would stall the whole ring).
                    add_dep_helper(bi.ins, chain[-1].ins, sync=False,
                                   reason="ring order")
                chain.append(bi)
                return bi

            for i in range(N_TILES):
                t = pool.tile([P, D], f32)
                tiles.append(t)
                rd = nc.sync if i % 2 == 0 else nc.scalar
                ring_dma(rd, t[:], x_ext[i * P : (i + 1) * P, :])
                nc.vector.tensor_mul(out=t[:], in0=t[:], in1=s_tile[:])
                j = i - WRITE_LAG
                if j >= 0:
                    wr = nc.sync if j % 2 == 0 else nc.scalar
                    ring_dma(wr, out_ext[j * P : (j + 1) * P, :], tiles[j][:])
            for j in range(N_TILES - WRITE_LAG, N_TILES):
                wr = nc.sync if j % 2 == 0 else nc.scalar
                ring_dma(wr, out_ext[j * P : (j + 1) * P, :], tiles[j][:])
    nc.finalize()
    return nc


def host_scale(alpha, omega, beta, eta) -> np.ndarray:
    h = omega.shape[0]
    j = np.arange(h, dtype=np.float64)
    theta = j[:, None] * omega[None, :].astype(np.float64) * (2.0 * math.pi / h)
    ct = np.cos(theta)
    ab = alpha.astype(np.float64) * beta.astype(np.float64)
    scale = np.einsum("dj,jk,dk->d", eta.astype(np.float64), ct, ab)
    return scale.astype(np.float32)


def run(x, scale, trace=False, tmpdir=None):
    nc = build_nc()
    scale_b = np.ascontiguousarray(np.broadcast_to(scale[None, :], (P, D)))
    in_maps = [
        {"x": np.ascontiguousarray(x[c * SHARD : (c + 1) * SHARD]), "scale": scale_b}
        for c in range(N_CORES)
    ]
    res = run_bass_kernel_spmd(
        nc, in_maps, core_ids=list(range(N_CORES)), trace=trace, tmpdir=tmpdir
    )
    out = np.concatenate([res.results[c]["out"] for c in range(N_CORES)], axis=0)
    return out, res


def kernel(x, alpha, delta, omega, beta, eta):
    x = np.asarray(x, dtype=np.float32)
    scale = host_scale(
        np.asarray(alpha), np.asarray(omega), np.asarray(beta), np.asarray(eta)
    )
    out, _ = run(x, scale)
    return out


# revision 22
# speedup vs baseline: 1.1228x; 1.1228x over previous
"""Distributed Trainium2 kernel for nn_CEMA_34445637714419.

Math (from the reference):
    scale[d] = sum_{j,k} eta[d,j] * cos(j*omega[k]*2pi/h) * alpha[d,k] * beta[d,k]
    y[b,d]   = x[b,d] * scale[d]

The (d,) scale vector costs ~17 MFLOP — computed on host in float64.
The device kernel is the pure memory-bound part: stream x (16384,2048) f32
through SBUF, multiply by the partition-replicated scale row, stream out.
Sharding: x split along batch across 8 NeuronCores (data parallel),
scale replicated.

Measured HW model (trn2, this kernel):
  - Two HWDGE rings (SP, ACT), each ~212 GB/s, ~4 outstanding DMAs deep.
  - Mixed directions ACROSS rings sustain ~425 GB/s combined (fabric cap);
    mixing directions WITHIN a ring collapses to ~350.
  - Per-direction HBM cap ~340 GB/s; SWDGE ring ~208 GB/s, slow spin-up.
So: reads stream on SP, writes on ACT, equal bytes per ring; the scale is
broadcast on-chip (K=1 PE matmul against ones) from an 8 KiB read instead
of burning a 1 MiB replicated read; the last tile is tapered so the final
read->mul->write dependency chain is short.
"""

import math
from contextlib import ExitStack

import numpy as np

try:
    import concourse.bass as bass
except ImportError:  # grading container may not have it on sys.path yet
    import sys

    sys.path.insert(0, "/opt/trn_rl_repo")
    import concourse.bass as bass

import concourse.bacc as bacc
import concourse.mybir as mybir
from concourse.bass_utils import run_bass_kernel_spmd
from concourse.tile import TileContext

BATCH = 16384
D = 2048
H = 64
N_CORES = 8
SHARD = BATCH // N_CORES  # 2048 rows per core
P = 128  # SBUF partitions
N_TILES = SHARD // P  # 16 tiles of (128, 2048) = 1 MiB each


def build_nc() -> bacc.Bacc:
    nc = bacc.Bacc(
        "TRN2", target_bir_lowering=False, debug=False, num_devices=N_CORES
    )
    f32 = mybir.dt.float32
    x_ext = nc.declare_dram_parameter("x", [SHARD, D], f32, isOutput=False)
    s_ext = nc.declare_dram_parameter("scale", [1, D], f32, isOutput=False)
    out_ext = nc.declare_dram_parameter("out", [SHARD, D], f32, isOutput=True)

    # Column taper of the last row-block: the final read->mul->write chain
    # shrinks from ~9 us (1 MiB granularity) to ~4 us.
    TAPER = [(0, 1024), (1024, 512), (1536, 256), (1792, 256)]

    with TileContext(nc) as tc, ExitStack() as ctx:
        with (
            tc.tile_pool(name="const", bufs=1) as cpool,
            tc.tile_pool(name="psum", bufs=1, space="PSUM") as ppool,
            # bufs == N_TILES: every x tile gets its own SBUF slot (16 MiB
            # total), so there is no slot reuse and no WAR/WAW waits — the
            # TensorTensor ISA slot only fits one semaphore wait.
            tc.tile_pool(name="io", bufs=N_TILES) as pool,
        ):
            s_row = cpool.tile([1, D], f32)
            ones_t = cpool.tile([1, P], f32)
            s_tile = cpool.tile([P, D], f32)
            ps = ppool.tile([P, D], f32)
            nc.sync.dma_start(s_row[:], s_ext[:])  # 8 KiB on the SP head
            nc.vector.memset(ones_t[:], 1.0)
            # Broadcast scale across partitions: ones(1,128).T @ s(1,512)
            # per 512-col PSUM bank. K=1, so values are exact.
            for c in range(0, D, 512):
                nc.tensor.matmul(
                    ps[:, c : c + 512],
                    ones_t[:],
                    s_row[:, c : c + 512],
                    start=True,
                    stop=True,
                )
            # One DVE copy PSUM->SBUF: absorbs the PE dependency so every
            # tensor_mul below needs only its own x-DMA wait.
            nc.vector.tensor_copy(out=s_tile[:], in_=ps[:])

            tiles = []
            for i in range(N_TILES):
                t = pool.tile([P, D], f32)
                tiles.append(t)
                rows = x_ext[i * P : (i + 1) * P, :]
                if i < N_TILES - 1:
                    nc.sync.dma_start(t[:], rows)
                    nc.vector.tensor_mul(out=t[:], in0=t[:], in1=s_tile[:])
                else:
                    for c0, w in TAPER:
                        nc.sync.dma_start(t[:, c0 : c0 + w], rows[:, c0 : c0 + w])
                        nc.vector.tensor_mul(
                            out=t[:, c0 : c0 + w],
                            in0=t[:, c0 : c0 + w],
                            in1=s_tile[:, c0 : c0 + w],
                        )
            for j in range(N_TILES):
                orows = out_ext[j * P : (j + 1) * P, :]
                if j < N_TILES - 1:
                    nc.scalar.dma_start(orows, tiles[j][:])
                else:
                    for c0, w in TAPER:
                        nc.scalar.dma_start(
                            orows[:, c0 : c0 + w], tiles[j][:, c0 : c0 + w]
                        )
    nc.finalize()
    return nc


def host_scale(alpha, omega, beta, eta) -> np.ndarray:
    h = omega.shape[0]
    j = np.arange(h, dtype=np.float64)
    theta = j[:, None] * omega[None, :].astype(np.float64) * (2.0 * math.pi / h)
    ct = np.cos(theta)
    ab = alpha.astype(np.float64) * beta.astype(np.float64)
    scale = np.einsum("dj,jk,dk->d", eta.astype(np.float64), ct, ab)
    return scale.astype(np.float32)


def run(x, scale, trace=False, tmpdir=None):
    nc = build_nc()
    scale_b = np.ascontiguousarray(scale[None, :])
    in_maps = [
        {"x": np.ascontiguousarray(x[c * SHARD : (c + 1) * SHARD]), "scale": scale_b}
        for c in range(N_CORES)
    ]
    res = run_bass_kernel_spmd(
        nc, in_maps, core_ids=list(range(N_CORES)), trace=trace, tmpdir=tmpdir
    )
    out = np.concatenate([res.results[c]["out"] for c in range(N_CORES)], axis=0)
    return out, res


def kernel(x, alpha, delta, omega, beta, eta):
    x = np.asarray(x, dtype=np.float32)
    scale = host_scale(
        np.asarray(alpha), np.asarray(omega), np.asarray(beta), np.asarray(eta)
    )
    out, _ = run(x, scale)
    return out
